# revision 34
# baseline (speedup 1.0000x reference)
"""Self-contained Trainium2 Bass kernel for deformable conv 2d.

kernel(x, offset, weight) -> out, matching the jax reference:
  x[2,256,64,64] f32, offset[2,18,64,64] f32, weight[256,256,3,3] f32
  -> out[2,256,64,64] f32 (KH=KW=3, stride=1, pad=1, dil=1, DG=1).

Runs SPMD on 8 NeuronCores, data-parallel: core = (batch, spatial quarter).
See build_core_kernel docstring for the device-side pipeline.
"""

import sys

for _p in ("/opt/trn_rl_repo",):
    if _p not in sys.path:
        sys.path.insert(0, _p)



import numpy as np
import ml_dtypes

import concourse.bass as bass
import concourse.mybir as mybir
import concourse.tile as tile

F32 = mybir.dt.float32
BF16 = mybir.dt.bfloat16
I32 = mybir.dt.int32

N, CIN, H, W = 2, 256, 64, 64
COUT = 256
KH = KW = 3
K = KH * KW
S = H * W            # 4096 output positions per batch
SLOC = S // 4        # 1024 per core
TPC = 8              # ts slots per tap (SLOC/128)
NT = K * TPC         # 72 slots of [128 samples]

AluOp = mybir.AluOpType


def build_core_kernel(nc, tc, outs, ins):
    """Emit the per-core kernel. ins/outs are dicts of DRAM APs."""
    import os
    from contextlib import ExitStack

    xi = ins["xi"]          # [4096, 512] bf16 y-pair-interleaved image
    wT = ins["wT"]          # [2304, 256] bf16 lhsT
    ridx_d = ins["ridx"]    # [128, 72] i32 gather rows (host-computed)
    cw_d = ins["cw"]        # [4, 128, 72] f32 corner weights (host-computed)
    ident = ins["ident"]    # [128, 128] bf16 identity
    out = outs["out"]       # [128, 2, 1024] f32

    ctx = ExitStack()
    gp = ctx.enter_context(tc.tile_pool(name="gather", bufs=16))
    cp = ctx.enter_context(tc.tile_pool(name="colsrow", bufs=8))
    rp = ctx.enter_context(tc.tile_pool(name="rhsT", bufs=1))
    pp = ctx.enter_context(tc.tile_pool(name="psum", bufs=1, space="PSUM"))
    tp = ctx.enter_context(tc.tile_pool(name="tpsum", bufs=4, space="PSUM"))
    sp = ctx.enter_context(tc.tile_pool(name="static", bufs=1))

    v = nc.vector

    # ---- static loads (gather indices first: they gate everything) ----
    ridx = sp.tile([128, NT], I32, name="ridx")
    nc.sync.dma_start(ridx[:], ridx_d)
    cwt = sp.tile([128, 4, NT], F32, name="cwt")
    nc.sync.dma_start(cwt[:], cw_d.rearrange("a p t -> p a t"))
    wT_s = sp.tile([128, 18, 256], BF16, name="wT_s")
    nc.sync.dma_start(wT_s[:], wT.rearrange("(j p) o -> p j o", p=128))
    id_s = sp.tile([128, 128], BF16, name="id_s")
    nc.sync.dma_start(id_s[:], ident)

    # ---- per-slot: gather + blend + transpose; per-tap matmuls ----
    rhsT = [rp.tile([128, 2, SLOC], BF16, name=f"rhsT{k}") for k in range(K)]
    ps = [pp.tile([128, SLOC], F32, name=f"psum{h}") for h in range(2)]
    osb = sp.tile([128, 2, SLOC], F32, name="osb")
    pts = []

    for t in range(NT):
        k, ts = t // TPC, t % TPC

        vt = gp.tile([128, 4, 256], BF16, name="vt")
        nc.gpsimd.indirect_dma_start(
            out=vt[:].rearrange("p a b -> p (a b)"),
            out_offset=None,
            in_=xi,
            in_offset=bass.IndirectOffsetOnAxis(ap=ridx[:, t : t + 1], axis=0),
        )

        # blend x-pairs: chain A on DVE, chain B's scale on Act (idle engine)
        cr = cp.tile([128, 256], BF16, name="colsrow")
        tm = cp.tile([128, 256], BF16, name="crtmp")
        v.tensor_scalar(cr[:], vt[:, 0, :], cwt[:, 0, t : t + 1], None, AluOp.mult)
        v.scalar_tensor_tensor(
            cr[:], vt[:, 2, :], cwt[:, 2, t : t + 1], cr[:], AluOp.mult, AluOp.add
        )
        v.tensor_scalar(tm[:], vt[:, 1, :], cwt[:, 1, t : t + 1], None, AluOp.mult)
        v.scalar_tensor_tensor(
            tm[:], vt[:, 3, :], cwt[:, 3, t : t + 1], tm[:], AluOp.mult, AluOp.add
        )
        # DVE paces the body: run the final add on the mostly idle GpSimd
        # engine for half the slots (tensor_tensor is the one elementwise op
        # walrus accepts on Pool). Keep tail slots on DVE (latency-critical).
        e = nc.gpsimd if (t % 2 == 0 and t < 68) else v
        e.tensor_tensor(cr[:], cr[:], tm[:], AluOp.add)

        tsl = ts % 4
        if tsl == 0:
            pt = tp.tile([128, 2, 4, 128], BF16, name="tpsum", space="PSUM")
            pts.append(pt)
        pt = pts[-1]
        for ch in range(2):
            nc.tensor.matmul(
                pt[:, ch, tsl, :], cr[:, ch * 128 : (ch + 1) * 128], id_s[:],
                is_transpose=True, start=True, stop=True,
            )
        if tsl == 3:
            ts0 = ts - 3
            nc.scalar.copy(
                rhsT[k][:, :, ts0 * 128 : (ts0 + 4) * 128].rearrange(
                    "p a (c b) -> p a c b", c=4
                ),
                pt[:],
            )

        if ts == TPC - 1:
            for h in range(2):
                for ch in range(2):
                    j = 2 * k + ch
                    for sh in range(2):
                        nc.tensor.matmul(
                            ps[h][:, sh * 512 : (sh + 1) * 512],
                            wT_s[:, j, h * 128 : (h + 1) * 128],
                            rhsT[k][:, ch, sh * 512 : (sh + 1) * 512],
                            start=(j == 0),
                            stop=(j == 17),
                        )
                if k == K - 1:
                    # stream each output half out as soon as its PSUM
                    # region finishes, overlapping the other half's matmuls
                    nc.scalar.copy(osb[:, h, :], ps[h][:])
                    nc.sync.dma_start(out[:, h, :], osb[:, h, :])

    ctx.close()


# ---------------- host-side prep ----------------

def core_inputs(x, offset, weight):
    """Full inputs (np f32) -> list of 8 per-core input dicts."""
    bf = ml_dtypes.bfloat16
    x = np.asarray(x, np.float32)
    offset = np.asarray(offset, np.float32)
    weight = np.asarray(weight, np.float32)

    # y-pair-interleaved channels-last images, bf16: xi[r] = [x[r], x[r+64]]
    xis = []
    for n in range(N):
        xcl = np.ascontiguousarray(x[n].reshape(CIN, S).T)  # [4096, 256]
        xi = np.zeros((S, 2 * CIN), np.float32)
        xi[:, :CIN] = xcl
        xi[: S - W, CIN:] = xcl[W:]
        xis.append(xi.astype(bf))

    # lhsT [k*256+c, o]
    wk = weight.reshape(COUT, CIN, K)           # [o, c, k]
    wT = np.ascontiguousarray(wk.transpose(2, 1, 0).reshape(K * CIN, COUT)).astype(bf)

    ident = np.eye(128, dtype=bf)

    off = offset.reshape(N, K, 2, S)
    ky, kx = np.meshgrid(np.arange(KH), np.arange(KW), indexing="ij")
    ky = ky.reshape(K, 1).astype(np.float32)
    kx = kx.reshape(K, 1).astype(np.float32)
    ho, wo = np.meshgrid(np.arange(H), np.arange(W), indexing="ij")
    base_y = ho.reshape(1, S).astype(np.float32) - 1.0 + ky   # [K, S]
    base_x = wo.reshape(1, S).astype(np.float32) - 1.0 + kx

    ins = []
    for core in range(8):
        n, qtr = core // 4, core % 4
        sl = slice(qtr * SLOC, (qtr + 1) * SLOC)
        py = base_y[:, sl] + off[n, :, 0, sl]   # [K, 1024]
        px = base_x[:, sl] + off[n, :, 1, sl]

        fy = np.floor(py)
        fx = np.floor(px)
        ly, lx = py - fy, px - fx
        hy, hx = 1.0 - ly, 1.0 - lx
        wy_c = np.clip(fy, 0.0, 62.0)           # window start rows
        wx_c = np.clip(fx, 0.0, 62.0)

        def sw(f, l, h, wc):
            """weights of window slots 0/1 along one axis, validity folded."""
            v0 = (f >= 0) & (f <= 63)
            v1 = (f + 1 >= 0) & (f + 1 <= 63)
            w0 = h * v0
            w1 = l * v1
            return [w0 * (wc + s_ == f) + w1 * (wc + s_ == f + 1) for s_ in (0, 1)]

        wys = sw(fy, ly, hy, wy_c)
        wxs = sw(fx, lx, hx, wx_c)
        rows = (wy_c * 64.0 + wx_c).astype(np.int32)        # [K, 1024]

        def lay(a):  # [K, 1024] -> [128, 72]: [p, k*8+ts] = a[k, p*8+ts]
            aq = np.asarray(a, np.float32).reshape(K, 128, TPC)   # [k, p, ts]
            return np.ascontiguousarray(
                aq.transpose(1, 0, 2).reshape(128, NT)
            )

        cwm = np.stack([
            lay(wys[0] * wxs[0]), lay(wys[1] * wxs[0]),
            lay(wys[0] * wxs[1]), lay(wys[1] * wxs[1]),
        ])                                                   # [4, 128, 72]

        ins.append({
            "xi": xis[n],
            "wT": wT,
            "ident": ident,
            "ridx": lay(rows).astype(np.int32),
            "cw": cwm,
        })
    return ins


def assemble(results):
    """list of 8 per-core {'out': [128,2,1024] f32} -> [2,256,64,64] f32."""
    out = np.zeros((N, COUT, S), np.float32)
    for core in range(8):
        n, qtr = core // 4, core % 4
        o = np.asarray(results[core]["out"])          # [128, 2, 1024]
        o = o.transpose(1, 0, 2).reshape(COUT, SLOC)  # [o, s'] s' = ts*128+p
        o = o.reshape(COUT, TPC, 128).transpose(0, 2, 1).reshape(COUT, SLOC)
        out[n, :, qtr * SLOC : (qtr + 1) * SLOC] = o
    return out.reshape(N, COUT, H, W)


def declare_io(nc):
    ins = {
        "xi": nc.dram_tensor("xi", [S, 2 * CIN], BF16, kind="ExternalInput").ap(),
        "wT": nc.dram_tensor("wT", [K * CIN, COUT], BF16, kind="ExternalInput").ap(),
        "ident": nc.dram_tensor("ident", [128, 128], BF16, kind="ExternalInput").ap(),
        "ridx": nc.dram_tensor("ridx", [128, NT], I32, kind="ExternalInput").ap(),
        "cw": nc.dram_tensor("cw", [4, 128, NT], F32, kind="ExternalInput").ap(),
    }
    outs = {
        "out": nc.dram_tensor("out", [128, 2, SLOC], F32, kind="ExternalOutput").ap(),
    }
    return outs, ins


def build_module():
    from concourse import bacc

    nc = bacc.Bacc(
        "TRN2",
        target_bir_lowering=False,
        debug=False,
        num_devices=8,
        dynamic_dma_scratch_size=65536,
    )
    outs, ins = declare_io(nc)
    with tile.TileContext(nc) as tc:
        build_core_kernel(nc, tc, outs, ins)
    nc.compile()
    return nc


_NC_CACHE = []


def kernel(x, offset, weight):
    """Full (unsharded) inputs -> full output, computed on 8 NeuronCores."""
    import time

    from concourse.bass_utils import run_bass_kernel_spmd

    if not _NC_CACHE:
        _NC_CACHE.append(build_module())
    nc = _NC_CACHE[0]
    core_ins = core_inputs(x, offset, weight)
    last = None
    for attempt in range(3):
        try:
            res = run_bass_kernel_spmd(nc, core_ins, core_ids=list(range(8)))
            return assemble(res.results)
        except Exception as e:  # transient device-session failures
            last = e
            time.sleep(2.0 * (attempt + 1))
    raise last



# revision 40
# speedup vs baseline: 1.0278x; 1.0278x over previous
"""Self-contained Trainium2 Bass kernel for deformable conv 2d.

kernel(x, offset, weight) -> out, matching the jax reference:
  x[2,256,64,64] f32, offset[2,18,64,64] f32, weight[256,256,3,3] f32
  -> out[2,256,64,64] f32 (KH=KW=3, stride=1, pad=1, dil=1, DG=1).

Runs SPMD on 8 NeuronCores, data-parallel: core = (batch, spatial quarter).
See build_core_kernel docstring for the device-side pipeline.
"""

import sys

for _p in ("/opt/trn_rl_repo",):
    if _p not in sys.path:
        sys.path.insert(0, _p)



import numpy as np
import ml_dtypes

import concourse.bass as bass
import concourse.mybir as mybir
import concourse.tile as tile

F32 = mybir.dt.float32
BF16 = mybir.dt.bfloat16
I32 = mybir.dt.int32

N, CIN, H, W = 2, 256, 64, 64
COUT = 256
KH = KW = 3
K = KH * KW
S = H * W            # 4096 output positions per batch
SLOC = S // 4        # 1024 per core
TPC = 8              # ts slots per tap (SLOC/128)
NT = K * TPC         # 72 slots of [128 samples]

AluOp = mybir.AluOpType


def build_core_kernel(nc, tc, outs, ins):
    """Emit the per-core kernel. ins/outs are dicts of DRAM APs."""
    import os
    from contextlib import ExitStack

    xi = ins["xi"]          # [4096, 512] bf16 y-pair-interleaved image
    wT = ins["wT"]          # [2304, 256] bf16 lhsT
    ridx_d = ins["ridx"]    # [128, 72] i32 gather rows (host-computed)
    cw_d = ins["cw"]        # [4, 128, 72] f32 corner weights (host-computed)
    ident = ins["ident"]    # [128, 128] bf16 identity
    out = outs["out"]       # [128, 2, 1024] f32

    ctx = ExitStack()
    gp = ctx.enter_context(tc.tile_pool(name="gather", bufs=16))
    cp = ctx.enter_context(tc.tile_pool(name="colsrow", bufs=14))
    rp = ctx.enter_context(tc.tile_pool(name="rhsT", bufs=1))
    pp = ctx.enter_context(tc.tile_pool(name="psum", bufs=1, space="PSUM"))
    tp = ctx.enter_context(tc.tile_pool(name="tpsum", bufs=4, space="PSUM"))
    sp = ctx.enter_context(tc.tile_pool(name="static", bufs=1))

    v = nc.vector

    # ---- static loads (gather indices first: they gate everything) ----
    ridx = sp.tile([128, NT], I32, name="ridx")
    nc.sync.dma_start(ridx[:], ridx_d)
    cwt = sp.tile([128, 4, NT], F32, name="cwt")
    nc.sync.dma_start(cwt[:], cw_d.rearrange("a p t -> p a t"))
    wT_s = sp.tile([128, 18, 256], BF16, name="wT_s")
    nc.sync.dma_start(wT_s[:], wT.rearrange("(j p) o -> p j o", p=128))
    id_s = sp.tile([128, 128], BF16, name="id_s")
    nc.sync.dma_start(id_s[:], ident)

    # ---- per-slot: gather + blend + transpose; per-tap matmuls ----
    rhsT = [rp.tile([128, 2, SLOC], BF16, name=f"rhsT{k}") for k in range(K)]
    ps = [pp.tile([128, SLOC], F32, name=f"psum{h}") for h in range(2)]
    osb = sp.tile([128, 2, SLOC], F32, name="osb")
    pts = []

    # DVE paces the body: the final blend add of every other slot runs on the
    # mostly idle GpSimd engine. To keep Pool's in-order queue from blocking
    # gather triggers, each offloaded add is emitted D slots late (its DVE
    # inputs are resolved by then), and the consume side (transposes, psum
    # copies, conv) is uniformly deferred by D slots so emission order still
    # matches data order.
    D = 4
    pend_add = {}
    slot_cr = []

    for t in range(NT + D):
        if t < NT:
            # deferred Pool add for slot t-D, before this slot's trigger
            if t - D in pend_add:
                pcr, ptm = pend_add.pop(t - D)
                nc.gpsimd.tensor_tensor(pcr[:], pcr[:], ptm[:], AluOp.add)

            vt = gp.tile([128, 4, 256], BF16, name="vt")
            nc.gpsimd.indirect_dma_start(
                out=vt[:].rearrange("p a b -> p (a b)"),
                out_offset=None,
                in_=xi,
                in_offset=bass.IndirectOffsetOnAxis(ap=ridx[:, t : t + 1], axis=0),
            )

            # blend x-pairs on DVE (two independent 2-op chains + final add)
            cr = cp.tile([128, 256], BF16, name="colsrow")
            tm = cp.tile([128, 256], BF16, name="crtmp")
            v.tensor_scalar(cr[:], vt[:, 0, :], cwt[:, 0, t : t + 1], None, AluOp.mult)
            v.scalar_tensor_tensor(
                cr[:], vt[:, 2, :], cwt[:, 2, t : t + 1], cr[:], AluOp.mult, AluOp.add
            )
            v.tensor_scalar(tm[:], vt[:, 1, :], cwt[:, 1, t : t + 1], None, AluOp.mult)
            v.scalar_tensor_tensor(
                tm[:], vt[:, 3, :], cwt[:, 3, t : t + 1], tm[:], AluOp.mult, AluOp.add
            )
            if t % 2 == 0 and t < NT - TPC:
                pend_add[t] = (cr, tm)   # tail slots stay on DVE
            else:
                v.tensor_tensor(cr[:], cr[:], tm[:], AluOp.add)
            slot_cr.append(cr)

        u = t - D
        if u < 0:
            continue
        k, ts = u // TPC, u % TPC
        cru = slot_cr[u]

        tsl = ts % 4
        if tsl == 0:
            pt = tp.tile([128, 2, 4, 128], BF16, name="tpsum", space="PSUM")
            pts.append(pt)
        pt = pts[-1]
        for ch in range(2):
            nc.tensor.matmul(
                pt[:, ch, tsl, :], cru[:, ch * 128 : (ch + 1) * 128], id_s[:],
                is_transpose=True, start=True, stop=True,
            )
        if tsl == 3:
            ts0 = ts - 3
            nc.scalar.copy(
                rhsT[k][:, :, ts0 * 128 : (ts0 + 4) * 128].rearrange(
                    "p a (c b) -> p a c b", c=4
                ),
                pt[:],
            )

        if ts == TPC - 1:
            for h in range(2):
                for ch in range(2):
                    j = 2 * k + ch
                    for sh in range(2):
                        nc.tensor.matmul(
                            ps[h][:, sh * 512 : (sh + 1) * 512],
                            wT_s[:, j, h * 128 : (h + 1) * 128],
                            rhsT[k][:, ch, sh * 512 : (sh + 1) * 512],
                            start=(j == 0),
                            stop=(j == 17),
                        )
                if k == K - 1:
                    # stream each output half out as soon as its PSUM
                    # region finishes, overlapping the other half's matmuls
                    nc.scalar.copy(osb[:, h, :], ps[h][:])
                    nc.sync.dma_start(out[:, h, :], osb[:, h, :])

    ctx.close()


# ---------------- host-side prep ----------------

def core_inputs(x, offset, weight):
    """Full inputs (np f32) -> list of 8 per-core input dicts."""
    bf = ml_dtypes.bfloat16
    x = np.asarray(x, np.float32)
    offset = np.asarray(offset, np.float32)
    weight = np.asarray(weight, np.float32)

    # y-pair-interleaved channels-last images, bf16: xi[r] = [x[r], x[r+64]]
    xis = []
    for n in range(N):
        xcl = np.ascontiguousarray(x[n].reshape(CIN, S).T)  # [4096, 256]
        xi = np.zeros((S, 2 * CIN), np.float32)
        xi[:, :CIN] = xcl
        xi[: S - W, CIN:] = xcl[W:]
        xis.append(xi.astype(bf))

    # lhsT [k*256+c, o]
    wk = weight.reshape(COUT, CIN, K)           # [o, c, k]
    wT = np.ascontiguousarray(wk.transpose(2, 1, 0).reshape(K * CIN, COUT)).astype(bf)

    ident = np.eye(128, dtype=bf)

    off = offset.reshape(N, K, 2, S)
    ky, kx = np.meshgrid(np.arange(KH), np.arange(KW), indexing="ij")
    ky = ky.reshape(K, 1).astype(np.float32)
    kx = kx.reshape(K, 1).astype(np.float32)
    ho, wo = np.meshgrid(np.arange(H), np.arange(W), indexing="ij")
    base_y = ho.reshape(1, S).astype(np.float32) - 1.0 + ky   # [K, S]
    base_x = wo.reshape(1, S).astype(np.float32) - 1.0 + kx

    ins = []
    for core in range(8):
        n, qtr = core // 4, core % 4
        sl = slice(qtr * SLOC, (qtr + 1) * SLOC)
        py = base_y[:, sl] + off[n, :, 0, sl]   # [K, 1024]
        px = base_x[:, sl] + off[n, :, 1, sl]

        fy = np.floor(py)
        fx = np.floor(px)
        ly, lx = py - fy, px - fx
        hy, hx = 1.0 - ly, 1.0 - lx
        wy_c = np.clip(fy, 0.0, 62.0)           # window start rows
        wx_c = np.clip(fx, 0.0, 62.0)

        def sw(f, l, h, wc):
            """weights of window slots 0/1 along one axis, validity folded."""
            v0 = (f >= 0) & (f <= 63)
            v1 = (f + 1 >= 0) & (f + 1 <= 63)
            w0 = h * v0
            w1 = l * v1
            return [w0 * (wc + s_ == f) + w1 * (wc + s_ == f + 1) for s_ in (0, 1)]

        wys = sw(fy, ly, hy, wy_c)
        wxs = sw(fx, lx, hx, wx_c)
        rows = (wy_c * 64.0 + wx_c).astype(np.int32)        # [K, 1024]

        def lay(a):  # [K, 1024] -> [128, 72]: [p, k*8+ts] = a[k, p*8+ts]
            aq = np.asarray(a, np.float32).reshape(K, 128, TPC)   # [k, p, ts]
            return np.ascontiguousarray(
                aq.transpose(1, 0, 2).reshape(128, NT)
            )

        cwm = np.stack([
            lay(wys[0] * wxs[0]), lay(wys[1] * wxs[0]),
            lay(wys[0] * wxs[1]), lay(wys[1] * wxs[1]),
        ])                                                   # [4, 128, 72]

        ins.append({
            "xi": xis[n],
            "wT": wT,
            "ident": ident,
            "ridx": lay(rows).astype(np.int32),
            "cw": cwm,
        })
    return ins


def assemble(results):
    """list of 8 per-core {'out': [128,2,1024] f32} -> [2,256,64,64] f32."""
    out = np.zeros((N, COUT, S), np.float32)
    for core in range(8):
        n, qtr = core // 4, core % 4
        o = np.asarray(results[core]["out"])          # [128, 2, 1024]
        o = o.transpose(1, 0, 2).reshape(COUT, SLOC)  # [o, s'] s' = ts*128+p
        o = o.reshape(COUT, TPC, 128).transpose(0, 2, 1).reshape(COUT, SLOC)
        out[n, :, qtr * SLOC : (qtr + 1) * SLOC] = o
    return out.reshape(N, COUT, H, W)


def declare_io(nc):
    ins = {
        "xi": nc.dram_tensor("xi", [S, 2 * CIN], BF16, kind="ExternalInput").ap(),
        "wT": nc.dram_tensor("wT", [K * CIN, COUT], BF16, kind="ExternalInput").ap(),
        "ident": nc.dram_tensor("ident", [128, 128], BF16, kind="ExternalInput").ap(),
        "ridx": nc.dram_tensor("ridx", [128, NT], I32, kind="ExternalInput").ap(),
        "cw": nc.dram_tensor("cw", [4, 128, NT], F32, kind="ExternalInput").ap(),
    }
    outs = {
        "out": nc.dram_tensor("out", [128, 2, SLOC], F32, kind="ExternalOutput").ap(),
    }
    return outs, ins


def build_module():
    from concourse import bacc

    nc = bacc.Bacc(
        "TRN2",
        target_bir_lowering=False,
        debug=False,
        num_devices=8,
        dynamic_dma_scratch_size=65536,
    )
    outs, ins = declare_io(nc)
    with tile.TileContext(nc) as tc:
        build_core_kernel(nc, tc, outs, ins)
    nc.compile()
    return nc


_NC_CACHE = []


def kernel(x, offset, weight):
    """Full (unsharded) inputs -> full output, computed on 8 NeuronCores."""
    import time

    from concourse.bass_utils import run_bass_kernel_spmd

    if not _NC_CACHE:
        _NC_CACHE.append(build_module())
    nc = _NC_CACHE[0]
    core_ins = core_inputs(x, offset, weight)
    last = None
    for attempt in range(3):
        try:
            res = run_bass_kernel_spmd(nc, core_ins, core_ids=list(range(8)))
            return assemble(res.results)
        except Exception as e:  # transient device-session failures
            last = e
            time.sleep(2.0 * (attempt + 1))
    raise last



# revision 46
# speedup vs baseline: 1.2048x; 1.1722x over previous
"""Self-contained Trainium2 Bass kernel for deformable conv 2d.

kernel(x, offset, weight) -> out, matching the jax reference:
  x[2,256,64,64] f32, offset[2,18,64,64] f32, weight[256,256,3,3] f32
  -> out[2,256,64,64] f32 (KH=KW=3, stride=1, pad=1, dil=1, DG=1).

Runs SPMD on 8 NeuronCores, data-parallel: core = (batch, spatial quarter).
See build_core_kernel docstring for the device-side pipeline.
"""

import sys

for _p in ("/opt/trn_rl_repo",):
    if _p not in sys.path:
        sys.path.insert(0, _p)



import numpy as np
import ml_dtypes

import concourse.bass as bass
import concourse.mybir as mybir
import concourse.tile as tile

F32 = mybir.dt.float32
BF16 = mybir.dt.bfloat16
I32 = mybir.dt.int32

N, CIN, H, W = 2, 256, 64, 64
COUT = 256
KH = KW = 3
K = KH * KW
S = H * W            # 4096 output positions per batch
SLOC = S // 4        # 1024 per core
TPC = 8              # ts slots per tap (SLOC/128)
NT = K * TPC         # 72 slots of [128 samples]

AluOp = mybir.AluOpType


def build_core_kernel(nc, tc, outs, ins):
    """Emit the per-core kernel. ins/outs are dicts of DRAM APs."""
    import os
    from contextlib import ExitStack

    xi = ins["xi"]          # [4096, 512] bf16 y-pair-interleaved image
    wT = ins["wT"]          # [2304, 256] bf16 lhsT
    ridx_d = ins["ridx"]    # [128, 72] i32 gather rows (host-computed)
    cw_d = ins["cw"]        # [4, 128, 72] f32 corner weights (host-computed)
    ident = ins["ident"]    # [128, 128] bf16 identity
    out = outs["out"]       # [128, 2, 1024] f32

    ctx = ExitStack()
    gp = ctx.enter_context(tc.tile_pool(name="gather", bufs=16))
    cp = ctx.enter_context(tc.tile_pool(name="colsrow", bufs=8))
    rp = ctx.enter_context(tc.tile_pool(name="rhsT", bufs=1))
    pp = ctx.enter_context(tc.tile_pool(name="psum", bufs=1, space="PSUM"))
    tp = ctx.enter_context(tc.tile_pool(name="tpsum", bufs=4, space="PSUM"))
    sp = ctx.enter_context(tc.tile_pool(name="static", bufs=1))

    v = nc.vector

    # ---- static loads (gather indices first: they gate everything) ----
    ridx = sp.tile([128, NT], I32, name="ridx")
    nc.sync.dma_start(ridx[:], ridx_d)
    cwt = sp.tile([128, 4, NT], F32, name="cwt")
    nc.sync.dma_start(cwt[:], cw_d.rearrange("a p t -> p a t"))
    wT_s = sp.tile([128, 18, 256], BF16, name="wT_s")
    nc.sync.dma_start(wT_s[:], wT.rearrange("(j p) o -> p j o", p=128))
    id_s = sp.tile([128, 128], BF16, name="id_s")
    nc.sync.dma_start(id_s[:], ident)

    # ---- per-slot: gather + blend + transpose; per-tap matmuls ----
    # rhsT holds BOTH chain results (w=0: cr, w=1: tm); the pair-sum happens
    # inside the conv contraction (same weight chunk contracted against both)
    rhsT = [rp.tile([128, 2, 2, SLOC], BF16, name=f"rhsT{k}") for k in range(K)]
    ps = [pp.tile([128, SLOC], F32, name=f"psum{h}") for h in range(2)]
    osb = sp.tile([128, 2, SLOC], F32, name="osb")
    pts = []

    for t in range(NT):
        k, ts = t // TPC, t % TPC

        vt = gp.tile([128, 4, 256], BF16, name="vt")
        nc.gpsimd.indirect_dma_start(
            out=vt[:].rearrange("p a b -> p (a b)"),
            out_offset=None,
            in_=xi,
            in_offset=bass.IndirectOffsetOnAxis(ap=ridx[:, t : t + 1], axis=0),
        )

        # blend x-pairs: chain A on DVE, chain B's scale on Act (idle engine)
        cr = cp.tile([128, 256], BF16, name="colsrow")
        tm = cp.tile([128, 256], BF16, name="crtmp")
        v.tensor_scalar(cr[:], vt[:, 0, :], cwt[:, 0, t : t + 1], None, AluOp.mult)
        v.scalar_tensor_tensor(
            cr[:], vt[:, 2, :], cwt[:, 2, t : t + 1], cr[:], AluOp.mult, AluOp.add
        )
        v.tensor_scalar(tm[:], vt[:, 1, :], cwt[:, 1, t : t + 1], None, AluOp.mult)
        v.scalar_tensor_tensor(
            tm[:], vt[:, 3, :], cwt[:, 3, t : t + 1], tm[:], AluOp.mult, AluOp.add
        )
        # no DVE add: transpose BOTH chain results; the conv sums them

        sl2 = ts % 2
        if sl2 == 0:
            pt = tp.tile([128, 2, 4, 128], BF16, name="tpsum", space="PSUM")
            pts.append(pt)
        pt = pts[-1]
        for ch in range(2):
            for w, src in ((0, cr), (1, tm)):
                nc.tensor.matmul(
                    pt[:, ch, w * 2 + sl2, :],
                    src[:, ch * 128 : (ch + 1) * 128], id_s[:],
                    is_transpose=True, start=True, stop=True,
                )
        if sl2 == 1:
            ts0 = ts - 1
            nc.scalar.copy(
                rhsT[k][:, :, :, ts0 * 128 : (ts0 + 2) * 128].rearrange(
                    "p c w (s b) -> p c w s b", s=2
                ),
                pt[:].rearrange("p c (w s) b -> p c w s b", w=2),
            )

        if ts == TPC - 1:
            for h in range(2):
                for ch in range(2):
                    j = 2 * k + ch
                    for w in range(2):
                        for sh in range(2):
                            nc.tensor.matmul(
                                ps[h][:, sh * 512 : (sh + 1) * 512],
                                wT_s[:, j, h * 128 : (h + 1) * 128],
                                rhsT[k][:, ch, w, sh * 512 : (sh + 1) * 512],
                                start=(j == 0 and w == 0),
                                stop=(j == 17 and w == 1),
                            )
                if k == K - 1:
                    # stream each output half out as soon as its PSUM
                    # region finishes, overlapping the other half's matmuls
                    nc.scalar.copy(osb[:, h, :], ps[h][:])
                    nc.sync.dma_start(out[:, h, :], osb[:, h, :])

    ctx.close()


# ---------------- host-side prep ----------------

def core_inputs(x, offset, weight):
    """Full inputs (np f32) -> list of 8 per-core input dicts."""
    bf = ml_dtypes.bfloat16
    x = np.asarray(x, np.float32)
    offset = np.asarray(offset, np.float32)
    weight = np.asarray(weight, np.float32)

    # y-pair-interleaved channels-last images, bf16: xi[r] = [x[r], x[r+64]]
    xis = []
    for n in range(N):
        xcl = np.ascontiguousarray(x[n].reshape(CIN, S).T)  # [4096, 256]
        xi = np.zeros((S, 2 * CIN), np.float32)
        xi[:, :CIN] = xcl
        xi[: S - W, CIN:] = xcl[W:]
        xis.append(xi.astype(bf))

    # lhsT [k*256+c, o]
    wk = weight.reshape(COUT, CIN, K)           # [o, c, k]
    wT = np.ascontiguousarray(wk.transpose(2, 1, 0).reshape(K * CIN, COUT)).astype(bf)

    ident = np.eye(128, dtype=bf)

    off = offset.reshape(N, K, 2, S)
    ky, kx = np.meshgrid(np.arange(KH), np.arange(KW), indexing="ij")
    ky = ky.reshape(K, 1).astype(np.float32)
    kx = kx.reshape(K, 1).astype(np.float32)
    ho, wo = np.meshgrid(np.arange(H), np.arange(W), indexing="ij")
    base_y = ho.reshape(1, S).astype(np.float32) - 1.0 + ky   # [K, S]
    base_x = wo.reshape(1, S).astype(np.float32) - 1.0 + kx

    ins = []
    for core in range(8):
        n, qtr = core // 4, core % 4
        sl = slice(qtr * SLOC, (qtr + 1) * SLOC)
        py = base_y[:, sl] + off[n, :, 0, sl]   # [K, 1024]
        px = base_x[:, sl] + off[n, :, 1, sl]

        fy = np.floor(py)
        fx = np.floor(px)
        ly, lx = py - fy, px - fx
        hy, hx = 1.0 - ly, 1.0 - lx
        wy_c = np.clip(fy, 0.0, 62.0)           # window start rows
        wx_c = np.clip(fx, 0.0, 62.0)

        def sw(f, l, h, wc):
            """weights of window slots 0/1 along one axis, validity folded."""
            v0 = (f >= 0) & (f <= 63)
            v1 = (f + 1 >= 0) & (f + 1 <= 63)
            w0 = h * v0
            w1 = l * v1
            return [w0 * (wc + s_ == f) + w1 * (wc + s_ == f + 1) for s_ in (0, 1)]

        wys = sw(fy, ly, hy, wy_c)
        wxs = sw(fx, lx, hx, wx_c)
        rows = (wy_c * 64.0 + wx_c).astype(np.int32)        # [K, 1024]

        def lay(a):  # [K, 1024] -> [128, 72]: [p, k*8+ts] = a[k, p*8+ts]
            aq = np.asarray(a, np.float32).reshape(K, 128, TPC)   # [k, p, ts]
            return np.ascontiguousarray(
                aq.transpose(1, 0, 2).reshape(128, NT)
            )

        cwm = np.stack([
            lay(wys[0] * wxs[0]), lay(wys[1] * wxs[0]),
            lay(wys[0] * wxs[1]), lay(wys[1] * wxs[1]),
        ])                                                   # [4, 128, 72]

        ins.append({
            "xi": xis[n],
            "wT": wT,
            "ident": ident,
            "ridx": lay(rows).astype(np.int32),
            "cw": cwm,
        })
    return ins


def assemble(results):
    """list of 8 per-core {'out': [128,2,1024] f32} -> [2,256,64,64] f32."""
    out = np.zeros((N, COUT, S), np.float32)
    for core in range(8):
        n, qtr = core // 4, core % 4
        o = np.asarray(results[core]["out"])          # [128, 2, 1024]
        o = o.transpose(1, 0, 2).reshape(COUT, SLOC)  # [o, s'] s' = ts*128+p
        o = o.reshape(COUT, TPC, 128).transpose(0, 2, 1).reshape(COUT, SLOC)
        out[n, :, qtr * SLOC : (qtr + 1) * SLOC] = o
    return out.reshape(N, COUT, H, W)


def declare_io(nc):
    ins = {
        "xi": nc.dram_tensor("xi", [S, 2 * CIN], BF16, kind="ExternalInput").ap(),
        "wT": nc.dram_tensor("wT", [K * CIN, COUT], BF16, kind="ExternalInput").ap(),
        "ident": nc.dram_tensor("ident", [128, 128], BF16, kind="ExternalInput").ap(),
        "ridx": nc.dram_tensor("ridx", [128, NT], I32, kind="ExternalInput").ap(),
        "cw": nc.dram_tensor("cw", [4, 128, NT], F32, kind="ExternalInput").ap(),
    }
    outs = {
        "out": nc.dram_tensor("out", [128, 2, SLOC], F32, kind="ExternalOutput").ap(),
    }
    return outs, ins


def build_module():
    from concourse import bacc

    nc = bacc.Bacc(
        "TRN2",
        target_bir_lowering=False,
        debug=False,
        num_devices=8,
        dynamic_dma_scratch_size=65536,
    )
    outs, ins = declare_io(nc)
    with tile.TileContext(nc) as tc:
        build_core_kernel(nc, tc, outs, ins)
    nc.compile()
    return nc


_NC_CACHE = []


def kernel(x, offset, weight):
    """Full (unsharded) inputs -> full output, computed on 8 NeuronCores."""
    import time

    from concourse.bass_utils import run_bass_kernel_spmd

    if not _NC_CACHE:
        _NC_CACHE.append(build_module())
    nc = _NC_CACHE[0]
    core_ins = core_inputs(x, offset, weight)
    last = None
    for attempt in range(3):
        try:
            res = run_bass_kernel_spmd(nc, core_ins, core_ids=list(range(8)))
            return assemble(res.results)
        except Exception as e:  # transient device-session failures
            last = e
            time.sleep(2.0 * (attempt + 1))
    raise last

